# revision 69
# baseline (speedup 1.0000x reference)
"""Distributed single-head causal attention for TRN2 (8 NeuronCores).

Problem: x[B=4, T=4096, C=768], Wq/Wk/Wv[H=64, C] ->
  out[b,t,:] = softmax(causal(q k^T * C^-0.05)) @ v   (single head)

Sharding: core ci = (batch b = ci//2, interleave half h = ci%2). Each core
computes k/v for its whole batch (from streamed x[b]^T) and attention for
the 16 q-tiles {2m+h} -- the interleaved split balances the causal triangle
between the two cores of a batch to within ~3%.

All 8 cores run ONE graph (uniform SPMD); every per-core difference (which
q-columns, causal-mask content) is carried in per-core DRAM inputs, never
in instruction-stream structure or access-pattern offsets.

Device algorithm per core (transposed-attention layout, no collectives):
  kv^T[128,s] = [Wk.T|Wv.T]-stationary f16 matmuls over streamed x^T chunks
  q^T [128,t] = [Wq.T|0]-stationary f16 matmuls over host-gathered xq^T
                (rows 64..127 of q^T zeroed, so using the full kv^T [128,T]
                 as the S-matmul stationary contributes nothing from the v
                 rows -- keeps every matmul at K=128, which is 2x faster
                 than K=64 on the PE and keeps the HAM clock warm)
  V_aug[s,65] = PE-transpose of kv^T chunks, cols 64..127, plus a ones
                column that makes the O matmul accumulate the softmax
                denominator for free
  S^T chunks  = kv^T-chunk-stationary matmuls, two chunks packed per
                [128,1024] PSUM pair -> ONE exp per pair on ScalarE
  P^T (bf16)  = exp(scale*S^T + mask); additive f16 masks (0/-6e4, enough
                to underflow the exp to exactly 0) come from DRAM, applied
                only to the 4 diagonal-region chunks
  O^T[65,512] += V_aug-stationary bf16 matmuls (software-pipelined 3 pairs
                behind S so the PE never stalls on the exp)
  out         = O^T DMA'd back [65,512] per q-block; the divide by the
                softmax denominator (row 64) and the transpose to [t,64]
                happen on the host during the unshard (tiny: B*T*65 flops)
No row-max subtraction (masked scores stay in [-53,51]; exp is exact in
f32).  Precision: f16 q/k/x/W (same 10-bit mantissa as the PE's TF32 mode),
bf16 P/V, f32 everywhere else -> rel err 2.3e-3 vs the f32 reference.

DMA strategy: x^T is pre-tiled on the host so each 512-col s-block's six
128-row contraction chunks lie side by side in one [128, 3072] row-major
block.  Block 0 of each stream is DMA'd as six [128,512] chunks (the first
projections drip-feed off them as they land); later blocks move as
[128,1536] halves whose issue is gated by recycled-pool-tag rotation.
Each engine-issued DMA costs ~600-800ns of sequencer dispatch, so few+large
DMAs keep the dispatch off the critical path (72+ small DMAs serialized on
two queues cost ~40us in an earlier revision).  Queues are need-ordered:
sync carries wkv | xq0 chunks | wq+id | xq halves | out blocks, gpsimd
carries xt chunks/halves + mask.

Schedule: a software-pipelined stream of attention S/O chunk-pairs with the
projection/transpose work chopped into single-matmul "filler units" drained
between pairs, so the in-order PE queue always holds work that does not
depend on the exp chain (ScalarE is the binding rate in the biggest
q-block, and any PE idle also drops the DVFS clock).  A NWARM-matmul
scratch burst bridges the NEFF preamble -> first-data window and steps the
clock governor up.  The last q-block ships out[0:256] as soon as the last
full pair lands, and the diagonal tail runs per-chunk exps, keeping the
final drain chain short.

Measured (neuron-profile, whole NEFF, fast-DVFS device): ~80.4us; the same
binary measures ~95-96us when the attached device is in its 2.0GHz state.
Journey: 93.6us -> 80.4us via DMA batching + need-ordered queues, host-side
normalize/transpose, filler-unit scheduling, chunk-interleaved drip of the
first kv/q blocks across both DMA queues, f16 masks, warmup and output-tail
restructuring.  (Pair-wise AllGather of kv^T halves works numerically on
this stack but costs ~18us latency per gather -- abandoned.)
"""

import sys

for _p in ("/opt/trn_rl_repo",):
    if _p not in sys.path:
        sys.path.insert(0, _p)

import numpy as np

import concourse.bass as bass  # noqa: F401  (registers engine classes)
import concourse.tile as tile
from concourse import bacc, mybir
from concourse.bass_utils import run_bass_kernel_spmd

B, T, C, H = 4, 4096, 768, 64
NCORES = 8
SCALE = float(C ** (-0.05))
CCH = C // 128          # 6 contraction chunks
NSB = T // 512          # 8 s-blocks (kv projection granularity)
NSC = T // 128          # 32 s-chunks (attention granularity)
TQ = T // 2             # 2048 q columns per core
NQB = TQ // 512         # 4 q-blocks
NEG = -60000.0       # f16-exact; exp(scale*(S+NEG)) underflows to 0.0
                     # (masks ship as f16 to halve their DMA footprint)
NWARM = 4            # >=4 or the HAM clock governor never steps the PE up
                     # to 2.4GHz (measured: NWARM=2 -> whole run at 2.0GHz).
                     # Keep minimal: warmup matmuls issue at a ~577ns
                     # sequencer cadence, so each extra one delays the first
                     # real (data-ready) matmul by ~0.6us

F32 = mybir.dt.float32
BF16 = mybir.dt.bfloat16
F16 = mybir.dt.float16
EXP = mybir.ActivationFunctionType.Exp

_CACHE: dict = {}


def _install_ntff_hook():
    """Provide antenv.axon_hooks if the image lacks it, so
    run_bass_kernel_spmd(trace=True) can capture NTFF profiles under axon."""
    try:
        from antenv.axon_hooks import get_axon_ntff_profile_hook  # noqa: F401
        return  # already present
    except ImportError:
        pass
    import contextlib
    import ctypes
    import types

    so_path = "/opt/axon/libaxon_pjrt.so"
    mod = types.ModuleType("antenv.axon_hooks")
    _state = {"hook": None}
    mod.set_axon_ntff_profile_hook = lambda h: _state.__setitem__("hook", h)
    mod.get_axon_ntff_profile_hook = lambda: _state["hook"]
    try:
        lib = ctypes.CDLL(so_path)
        if hasattr(lib, "axon_start_nrt_profile"):
            lib.axon_start_nrt_profile.argtypes = [
                ctypes.POINTER(ctypes.c_int64), ctypes.c_size_t]
            lib.axon_start_nrt_profile.restype = ctypes.c_int64
            lib.axon_stop_nrt_profile.argtypes = [ctypes.c_char_p]
            lib.axon_stop_nrt_profile.restype = ctypes.c_int64

            @contextlib.contextmanager
            def _hook(output_dir, device_ids):
                import jax
                jax.devices()
                if device_ids:
                    ids = (ctypes.c_int64 * len(device_ids))(*device_ids)
                    rc = lib.axon_start_nrt_profile(ids, len(device_ids))
                else:
                    rc = lib.axon_start_nrt_profile(None, 0)
                if rc != 0:
                    raise RuntimeError(f"axon_start_nrt_profile rc={rc}")
                try:
                    yield
                finally:
                    n = lib.axon_stop_nrt_profile(str(output_dir).encode())
                    print(f"profile: {n} file(s) written to {output_dir}")

            _state["hook"] = _hook
    except OSError:
        pass
    import antenv
    sys.modules["antenv.axon_hooks"] = mod
    antenv.axon_hooks = mod


_install_ntff_hook()


def _build_graph():
    nc = bacc.Bacc("TRN2", target_bir_lowering=False, debug=False,
                   num_devices=NCORES)

    # host-pretiled inputs: each [128, 3072] row block = one s/q block's six
    # 128-row contraction chunks side by side.
    xt_d = nc.dram_tensor("xt", [NSB * 128, CCH * 512], F16,
                          kind="ExternalInput")
    xqt_d = nc.dram_tensor("xqt", [NQB * 128, CCH * 512], F16,
                           kind="ExternalInput")
    # wkv chunks (768) | wq chunks (768) | identity (128)
    wp_d = nc.dram_tensor("wp", [128, 1664], F16, kind="ExternalInput")
    mask_d = nc.dram_tensor("mask", [128, 4 * 256], F16, kind="ExternalInput")
    out_d = nc.dram_tensor("out", [NQB * 65, 512], F32, kind="ExternalOutput")

    from collections import deque

    with tile.TileContext(nc) as tc:
        with (
            tc.tile_pool(name="consts", bufs=1) as consts,
            tc.tile_pool(name="xqp", bufs=3) as xqp,
            tc.tile_pool(name="xsp", bufs=6) as xsp,
            tc.tile_pool(name="persist", bufs=1) as persist,
            tc.tile_pool(name="ptile", bufs=6) as ptile,
            tc.tile_pool(name="opost", bufs=2) as opost,
            tc.tile_pool(name="pskv", bufs=1, space="PSUM") as pskv,
            tc.tile_pool(name="psv", bufs=1, space="PSUM") as psv,
            tc.tile_pool(name="pss", bufs=2, space="PSUM") as pss,
            tc.tile_pool(name="pso", bufs=2, space="PSUM") as pso,
        ):
            # ---- input DMAs, need-ordered per queue (each trigger queue
            # delivers FIFO, so the first-needed data heads each queue).
            # Block 0 of each stream is chunked [128,512] so the very first
            # projection matmuls can start as soon as ~130KB has landed. ----
            wp_t = consts.tile([128, 1664], F16, tag="wp", name="wp_t")
            # split: wkv lands first (kv0 is the first consumer), wq+id next
            nc.sync.dma_start(wp_t[:, 0:768], wp_d.ap()[:, 0:768])
            nc.sync.dma_start(wp_t[:, 768:1664], wp_d.ap()[:, 768:1664])
            mask_t = consts.tile([128, 4 * 256], F16, tag="mask", name="mask_t")

            def wkv_t(c):
                return wp_t[:, c * 128:(c + 1) * 128]

            def wq_t(c):
                return wp_t[:, 768 + c * 128:768 + (c + 1) * 128]

            id_t = wp_t[:, 1536:1664]

            xs0c = []
            for c in range(CCH):
                t = persist.tile([128, 512], F16, tag=f"xs0c{c}",
                                 name=f"xs0c{c}")
                nc.gpsimd.dma_start(t[:], xt_d.ap()[0:128,
                                                    c * 512:(c + 1) * 512])
                xs0c.append(t)
            xq0c = []
            for c in range(CCH):
                t = persist.tile([128, 512], F16, tag=f"xq0c{c}",
                                 name=f"xq0c{c}")
                nc.sync.dma_start(t[:], xqt_d.ap()[0:128,
                                                   c * 512:(c + 1) * 512])
                xq0c.append(t)
            nc.gpsimd.dma_start(mask_t[:], mask_d.ap()[:, :])

            # remaining stream: [128,1536] halves; recycled pool tags bound
            # SBUF liveness (issue is gated on the half `bufs` ago being
            # fully consumed).
            xq_h = {}
            xs_h = {}
            for hx in range(2, 2 * NQB):
                t = xqp.tile([128, 1536], F16, tag="xq", name=f"xqh{hx}")
                nc.sync.dma_start(
                    t[:], xqt_d.ap()[(hx // 2) * 128:(hx // 2 + 1) * 128,
                                     (hx % 2) * 1536:(hx % 2 + 1) * 1536])
                xq_h[hx] = t
            for hx in range(2, 2 * NSB):
                t = xsp.tile([128, 1536], F16, tag="xs", name=f"xsh{hx}")
                nc.gpsimd.dma_start(
                    t[:], xt_d.ap()[(hx // 2) * 128:(hx // 2 + 1) * 128,
                                    (hx % 2) * 1536:(hx % 2 + 1) * 1536])
                xs_h[hx] = t

            def xq_rhs(qb, c):
                if qb == 0:
                    return xq0c[c][:]
                return xq_h[2 * qb + c // 3][:, (c % 3) * 512:
                                             (c % 3 + 1) * 512]

            def xs_rhs(sb, c):
                if sb == 0:
                    return xs0c[c][:]
                return xs_h[2 * sb + c // 3][:, (c % 3) * 512:
                                             (c % 3 + 1) * 512]

            # ---- warmup: preload Exp LUT + wake the PE clock while the
            # input DMAs stream (both write scratch that nothing reads) ----
            wsc = persist.tile([128, 512], F16, tag="wsc", name="wsc")
            nc.vector.memset(wsc[:], 0.25)
            wact = persist.tile([128, 64], F32, tag="wact", name="wact")
            nc.vector.memset(wact[:], 0.5)
            nc.scalar.activation(wact[:], wact[:], EXP, scale=SCALE)
            wps = pss.tile([128, 1024], F32, tag="s", name="wps")
            for wi in range(NWARM):
                nc.tensor.matmul(wps[:, 0:512], lhsT=wsc[:, 0:128],
                                 rhs=wsc[:], start=True, stop=True)

            # ---- persistent intermediates ----
            kvt = persist.tile([128, T], F16, tag="kvt", name="kvt")
            qt = persist.tile([128, TQ], F16, tag="qt", name="qt")
            nc.vector.memset(qt[64:128, :], 0.0)
            vaug = persist.tile([128, NSC * (H + 1)], BF16, tag="vaug",
                                name="vaug")
            # ones everywhere once; the V chunk copies overwrite cols 0..63
            # of each 65-block, leaving the denominator column at +64.
            nc.vector.memset(vaug[:], 1.0)

            # ---- projection / transpose work as schedulable units ----
            # Each unit is one PE instruction (plus its cheap DVE follow-up);
            # units are drained between attention S/O pairs so the PE always
            # has independent work while ScalarE chews on the exps.
            def q_units(qb):
                # q accumulates in the psv-pool bank (temporally disjoint
                # from the kv transposes) so a q block and a kv block can be
                # in flight concurrently -- the li=0 drip interleaves them
                # chunk-by-chunk to consume both DMA queues in parallel.
                hold = {}

                def mm(c):
                    def f():
                        if c == 0:
                            hold["ps"] = psv.tile([128, 512], F32, tag="v",
                                                  name=f"psq{qb}")
                        nc.tensor.matmul(hold["ps"][:], lhsT=wq_t(c),
                                         rhs=xq_rhs(qb, c),
                                         start=(c == 0), stop=(c == CCH - 1))
                    return f

                def fin():
                    nc.vector.tensor_copy(qt[0:H, qb * 512:(qb + 1) * 512],
                                          hold["ps"][0:H, :])
                return [mm(c) for c in range(CCH)] + [fin]

            def kv_units(sb):
                hold = {}

                def mm(c):
                    def f():
                        if c == 0:
                            hold["ps"] = pskv.tile([128, 512], F32, tag="kv",
                                                   name=f"pkv{sb}")
                        nc.tensor.matmul(hold["ps"][:], lhsT=wkv_t(c),
                                         rhs=xs_rhs(sb, c),
                                         start=(c == 0), stop=(c == CCH - 1))
                    return f

                def fin():
                    nc.vector.tensor_copy(kvt[:, sb * 512:(sb + 1) * 512],
                                          hold["ps"][:])

                def tr(k):
                    def f():
                        sc = sb * 4 + k
                        pv = psv.tile([128, 128], F16, tag="v",
                                      name=f"pv{sc}")
                        nc.tensor.transpose(
                            pv[:], kvt[:, sc * 128:(sc + 1) * 128], id_t)
                        nc.vector.tensor_copy(
                            vaug[:, sc * (H + 1):sc * (H + 1) + H],
                            pv[:, 64:128])
                    return f
                return ([mm(c) for c in range(CCH)] + [fin]
                        + [tr(k) for k in range(4)])

            def emit_att(li, fill_a, fill_b):
                po = pso.tile([H + 1, 512], F32, tag="o", name=f"po{li}")
                nfull = 8 * li + 4
                n = 8 * li + 8
                state = {}

                def emit_spair(p):
                    si0 = 2 * p
                    full = si0 < nfull
                    ps_ = pss.tile([128, 1024], F32, tag="s",
                                   name=f"ps{li}_{p}")
                    pp = ptile.tile([128, 1024], BF16, tag="p",
                                    name=f"pp{li}_{p}")
                    for j, si in enumerate((si0, si0 + 1)):
                        if full:
                            nc.tensor.matmul(
                                ps_[:, j * 512:(j + 1) * 512],
                                lhsT=kvt[:, si * 128:(si + 1) * 128],
                                rhs=qt[:, li * 512:(li + 1) * 512],
                                start=True, stop=True)
                            r = si - 8 * li
                            if r >= 0:
                                nc.vector.tensor_add(
                                    ps_[:, j * 512:j * 512 + 256],
                                    ps_[:, j * 512:j * 512 + 256],
                                    mask_t[:, r * 256:(r + 1) * 256])
                        else:
                            # diagonal tail: per-chunk S->mask->exp so the
                            # drain chain at block end stays short and each
                            # O chunk can start as soon as its exp lands
                            nc.tensor.matmul(
                                ps_[:, j * 256:(j + 1) * 256],
                                lhsT=kvt[:, si * 128:(si + 1) * 128],
                                rhs=qt[:, li * 512 + 256:(li + 1) * 512],
                                start=True, stop=True)
                            r = si - nfull
                            nc.vector.tensor_add(
                                ps_[:, j * 256:(j + 1) * 256],
                                ps_[:, j * 256:(j + 1) * 256],
                                mask_t[:, r * 256:(r + 1) * 256])
                            nc.scalar.activation(
                                pp[:, j * 256:(j + 1) * 256],
                                ps_[:, j * 256:(j + 1) * 256],
                                EXP, scale=SCALE)
                    if full:
                        nc.scalar.activation(pp[:, 0:1024], ps_[:, 0:1024],
                                             EXP, scale=SCALE)
                    state[p] = pp

                def emit_opair(p):
                    si0 = 2 * p
                    full = si0 < nfull
                    pp = state.pop(p)
                    for j, si in enumerate((si0, si0 + 1)):
                        if full:
                            nc.tensor.matmul(
                                po[:],
                                lhsT=vaug[:, si * (H + 1):(si + 1) * (H + 1)],
                                rhs=pp[:, j * 512:(j + 1) * 512],
                                start=(si == 0), stop=(si == n - 1),
                                skip_group_check=True)
                        else:
                            nc.tensor.matmul(
                                po[:, 256:512],
                                lhsT=vaug[:, si * (H + 1):(si + 1) * (H + 1)],
                                rhs=pp[:, j * 256:(j + 1) * 256],
                                start=False, stop=(si == n - 1),
                                skip_group_check=True)

                npairs = n // 2
                nfp = nfull // 2
                LA = 3
                fa = deque(fill_a)
                fb = deque(fill_b)

                def drain(dq, p, plast):
                    slots = plast - p
                    k = -(-len(dq) // slots) if slots > 0 else len(dq)
                    for _ in range(k):
                        dq.popleft()()

                # segment A: pairs with chunks < nfull (need sb <= 2li)
                for p in range(nfp):
                    emit_spair(p)
                    drain(fa, p, nfp - 1)
                    if p >= LA:
                        emit_opair(p - LA)
                while fa:
                    fa.popleft()()
                # segment B: tail pairs (need sb 2li+1, complete once fa
                # has been flushed)
                for p in range(nfp, npairs):
                    emit_spair(p)
                    drain(fb, p, npairs - 1)
                    if p >= LA:
                        emit_opair(p - LA)
                while fb:
                    fb.popleft()()
                # drain; on the last block, ship cols 0:256 as soon as the
                # last FULL pair lands (the partial diagonal pairs only
                # touch cols 256:512), so the final DMA chain is short.
                split = (li == NQB - 1)
                osb = opost.tile([H + 1, 512], F32, tag="osb", name=f"osb{li}")
                for p in range(max(0, npairs - LA), npairs):
                    emit_opair(p)
                    if split and p == nfp - 1:
                        nc.vector.tensor_copy(osb[:, 0:256], po[:, 0:256])
                        nc.sync.dma_start(
                            out_d.ap()[li * 65:(li + 1) * 65, 0:256],
                            osb[:, 0:256])
                if split:
                    nc.vector.tensor_copy(osb[:, 256:512], po[:, 256:512])
                    nc.sync.dma_start(
                        out_d.ap()[li * 65:(li + 1) * 65, 256:512],
                        osb[:, 256:512])
                else:
                    # O^T out via an SBUF bounce (DMA can't read PSUM): host
                    # divides by the denominator row and transposes during
                    # the unshard.
                    nc.vector.tensor_copy(osb[:], po[:])
                    nc.sync.dma_start(
                        out_d.ap()[li * 65:(li + 1) * 65, :], osb[:])

            # ---- schedule ----
            # li=0 is DMA-starved: run kv0/q0 directly (they drip-feed off
            # the chunked first-block DMAs).  From li>=1, kv(2li+1) fills
            # segment A and q(li+1)+kv(2li+2) fill segment B, so every
            # ScalarE-bound stretch has independent PE work interleaved.
            ku0 = kv_units(0)
            qu0 = q_units(0)
            # chunk-interleave kv0/q0 so the drip consumes both DMA queues'
            # arrivals in parallel (xs0 on gpsimd, xq0 on sync)
            for c in range(CCH):
                ku0[c]()
                qu0[c]()
            ku0[CCH]()            # kvt copy
            qu0[CCH]()            # qt copy
            for u in ku0[CCH + 1:]:
                u()               # V transposes
            emit_att(0, kv_units(1), q_units(1) + kv_units(2))
            emit_att(1, kv_units(3), q_units(2) + kv_units(4))
            emit_att(2, kv_units(5), q_units(3) + kv_units(6))
            emit_att(3, kv_units(7), [])

    nc.compile()
    return nc


def _host_inputs(x, Wq, Wk, Wv):
    """Build the 8 per-core input maps from the full problem inputs."""
    tri = np.where(np.arange(128)[:, None] <= np.arange(128)[None, :],
                   np.float16(0.0), np.float16(NEG))          # valid s<=t
    keep = np.zeros((128, 128), np.float16)
    full = np.full((128, 128), np.float16(NEG), np.float16)

    def blk(cmp):
        return tri if cmp == 0 else (keep if cmp < 0 else full)

    def chunk_cols(a):
        """[C, N] -> [N//512, 128, CCH*512]: per 512-col block, the six
        128-row chunks side by side."""
        n = a.shape[1]
        return np.ascontiguousarray(
            a.reshape(CCH, 128, n // 512, 512).transpose(2, 1, 0, 3)
            .reshape(n // 512 * 128, CCH * 512))

    wkv = np.concatenate([Wk.T, Wv.T], axis=1).astype(np.float16)  # [C, 128]
    wq = np.concatenate(
        [Wq.T.astype(np.float16), np.zeros((C, 64), np.float16)], axis=1)
    # wp: wkv chunks | wq chunks | identity  -> [128, 1664]
    wp = np.ascontiguousarray(np.concatenate(
        [wkv.reshape(CCH, 128, 128).transpose(1, 0, 2).reshape(128, 768),
         wq.reshape(CCH, 128, 128).transpose(1, 0, 2).reshape(128, 768),
         np.eye(128, dtype=np.float16)], axis=1))

    in_maps = []
    for ci in range(NCORES):
        b, h = divmod(ci, 2)
        xt = x[b].T.astype(np.float16)                           # [C, T]
        gtiles = [2 * m + h for m in range(16)]
        qcols = np.concatenate(
            [np.arange(g * 128, (g + 1) * 128) for g in gtiles])
        # mask[r, tc]: s-chunk (4li+r) vs t-tile (4li + h + 2*tc)
        mrows = []
        for r in range(4):
            mrows.append(np.concatenate(
                [blk(r - h), blk(r - 2 - h)], axis=1))           # [128, 256]
        mask = np.ascontiguousarray(np.concatenate(mrows, axis=1))
        in_maps.append({
            "xt": chunk_cols(xt), "xqt": chunk_cols(xt[:, qcols]),
            "wp": wp, "mask": mask,
        })
    return in_maps


def _run(x, Wq, Wk, Wv, trace=False, trace_cores=None):
    if "nc" not in _CACHE:
        _CACHE["nc"] = _build_graph()
    nc = _CACHE["nc"]
    in_maps = _host_inputs(np.asarray(x), np.asarray(Wq),
                           np.asarray(Wk), np.asarray(Wv))
    res = run_bass_kernel_spmd(nc, in_maps, core_ids=list(range(NCORES)),
                               trace=trace, trace_cores=trace_cores)
    out = np.empty((B, T, H), np.float32)
    for ci in range(NCORES):
        b, h = divmod(ci, 2)
        ot = np.asarray(res.results[ci]["out"]).reshape(NQB, 65, 512)
        # divide by the softmax denominator (row 64), back to [TQ, H]
        core_out = np.ascontiguousarray(
            (ot[:, :H, :] / ot[:, H:H + 1, :]).transpose(0, 2, 1)
        ).reshape(TQ, H)
        for m in range(16):
            g = 2 * m + h
            out[b, g * 128:(g + 1) * 128, :] = \
                core_out[m * 128:(m + 1) * 128, :]
    return out, res


def kernel(x, Wq, Wk, Wv):
    out, _ = _run(x, Wq, Wk, Wv, trace=False)
    return out
